# revision 15
# baseline (speedup 1.0000x reference)
"""Trainium2 Bass kernel for block-diagonal sparse attention (8 NeuronCores SPMD).

Problem: nn_AttentionHead (N=4096, DIM_IN=512, DQ=DK=128, 16 graphs of 256 nodes).
  q = x@Wq.T+bq; k = x@Wk.T+bk; v = x@Wv.T+bv
  a = where(block, qk/sqrt(dq), 0) + b + c; masked-softmax over block-diagonal
  out = (softmax(a)*keep) @ v

Key structural facts exploited:
  - Only the 16 diagonal 256x256 tiles of b/c/sparse_mask matter; the host
    slices them, combines bcm = b+c (masked entries -> -200 so exp gives 0),
    casts to bf16. HBM traffic is ~1.2MB/core instead of ~200MB.
  - Graphs are independent -> 2 graphs per core across 8 cores, zero cross-core
    communication (weights replicated).
  - The single per-core DMA engine drains the sync HW queue before the scalar
    HW queue, so inputs are laid out across the two queues in exact dependency
    order: [wqk | x-g0 | wv+I | x-g1] then [bc-g0 | bc-g1].  Compute streams
    behind the transfer instead of waiting for all input; each DMA trigger
    costs ~0.7us of engine time, so transfers are kept few and large.
  - bcm is added into the score PSUM by the PE itself via an identity-matmul
    accumulated onto the qk matmul, so the only post-processing is a single
    exp per graph straight out of the (single-bank) PSUM tile.
  - The denominator is obtained free by appending a ones-column to v in the PV
    matmul; the division happens on the HOST (outputs leave the chip
    unnormalized as [num | den] rows in bf16).
  - q/k/v biases never touch the chip when they are all zero (the actual
    inputs): out = num/den + bv is exact because sm @ (v0 + 1*bv^T) =
    sm@v0 + den*bv^T, and the bq/bk terms only shift softmax rows by
    constants.  A nonzero-bias graph variant is compiled only if needed.
  - 1/sqrt(dq) is folded into Wq host-side; everything is pre-cast to bf16.
  - The PE HAM clock-gate unthrottles 1.2->2.4GHz only after ~4us of gapless
    matmul activity, so narrow dummy warmup matmuls bridge the input-DMA
    phase at fine granularity; the real matmuls then run at full clock.
  - Per-graph pipelining: graph 0's projections/scores/exp/PV run while graph
    1's x/bc are still in flight.
"""

import math

import numpy as np
import ml_dtypes

import concourse.bass as bass
import concourse.mybir as mybir
import concourse.tile as tile
from concourse import bacc
from concourse.bass_utils import run_bass_kernel_spmd

# -------- problem constants (hardcoded per spec) --------
N = 4096
DIN = 512
DQ = 128           # == DK
NG = 16            # number of graphs
G = N // NG        # 256 nodes per graph
NCORES = 8
RPC = N // NCORES  # 512 rows per core
GPC = NG // NCORES  # 2 graphs per core
NT = RPC // 128    # 4 row-tiles of 128 per core
KO = DIN // 128    # 4 contraction tiles for the projections
VA = DQ + 1        # v augmented with a ones column (denominator trick)
SCALE = 1.0 / math.sqrt(DQ)
NEG = -200.0       # masked-entry sentinel; exp(-200 + |qk|max) == 0 in bf16
NWARM = 10         # wide PE HAM warmup matmuls (bridge to ~data arrival)

F32 = mybir.dt.float32
BF16 = mybir.dt.bfloat16

ACT = mybir.ActivationFunctionType
ALU = mybir.AluOpType

BF = ml_dtypes.bfloat16

WQK = 2 * KO * DQ        # wq | wk columns
WVI = KO * DQ + 128      # wv | identity columns

_CACHE: dict = {}


def build_nc(with_bias: bool) -> bass.Bass:
    """Build the per-core Bass graph (identical on all 8 cores)."""
    nc = bacc.Bacc(
        "TRN2",
        target_bir_lowering=False,
        debug=False,
        enable_asserts=False,
        num_devices=NCORES,
    )
    wqk_d = nc.dram_tensor("wqk", [128, WQK], BF16, kind="ExternalInput").ap()
    wvi_d = nc.dram_tensor("wvi", [128, WVI], BF16, kind="ExternalInput").ap()
    x_d = [
        nc.dram_tensor(f"x{g}", [128, KO, G], BF16, kind="ExternalInput").ap()
        for g in range(GPC)
    ]
    bc_d = [
        nc.dram_tensor(f"bc{g}", [128, 2 * G], BF16, kind="ExternalInput").ap()
        for g in range(GPC)
    ]
    if with_bias:
        bia_d = nc.dram_tensor("bias", [DQ, 2], F32, kind="ExternalInput").ap()
    out_d = nc.dram_tensor("out", [128, NT, VA], BF16, kind="ExternalOutput").ap()
    out_sb_t = nc.alloc_sbuf_tensor("out_sb", [128, NT, VA], BF16)

    with tile.TileContext(nc) as tc:
        with (
            tc.tile_pool(name="const", bufs=1) as cpool,
            tc.tile_pool(name="eq", bufs=2) as epool,
            tc.tile_pool(name="ps_proj", bufs=2, space="PSUM") as pp,
            tc.tile_pool(name="ps_v", bufs=2, space="PSUM") as pvp,
            tc.tile_pool(name="ps_s", bufs=2, space="PSUM") as ps,
            tc.tile_pool(name="ps_o", bufs=2, space="PSUM") as po,
        ):
            # warm tile on gpsimd (its preamble finishes first) so the PE
            # warmup starts as early as possible; only the lhsT columns need
            # defined data -- the rhs may read stale SBUF
            warm = cpool.tile([128, RPC], BF16)
            nc.gpsimd.memset(warm[:, 0:128], 1.0)

            # ---- input DMAs; the single DMA engine round-robins between the
            # two HW queues, so the effective arrival order is the zipper of
            # the two queue sequences: wqk||x0 first (enables the g0 q/k
            # projections), then wvi||x1, then bc0/bc1 for the score adds ----
            wqk = cpool.tile([128, WQK], BF16)
            nc.sync.dma_start(wqk[:], wqk_d)
            xs = [cpool.tile([128, KO, G], BF16, name=f"x{g}") for g in range(GPC)]
            nc.scalar.dma_start(xs[0][:], x_d[0])
            bcs = [
                cpool.tile([128, 2 * G], BF16, name=f"bc{g}") for g in range(GPC)
            ]
            nc.sync.dma_start(bcs[0][:], bc_d[0])
            wvi = cpool.tile([128, WVI], BF16)
            nc.sync.dma_start(wvi[:], wvi_d)
            nc.scalar.dma_start(xs[1][:], x_d[1])
            nc.scalar.dma_start(bcs[1][:], bc_d[1])
            if with_bias:
                bia = cpool.tile([128, 2], F32)
                nc.scalar.dma_start(bia[:], bia_d)

            def wsl(s, ko):  # weight slice for projection s, contraction ko
                if s < 2:
                    return wqk[:, (s * KO + ko) * DQ:(s * KO + ko + 1) * DQ]
                return wvi[:, ko * DQ:(ko + 1) * DQ]

            idn = wvi[:, KO * DQ:KO * DQ + 128]

            vna = cpool.tile([128, NT, VA], BF16)  # [j%128, j//128, d | 1]
            nc.vector.memset(vna[:, :, DQ:VA], 1.0)

            # ---- PE HAM warmup: narrow matmuls so real work preempts the
            # bridge at fine granularity once its data lands ----
            for _ in range(NWARM):
                wp = pp.tile([128, RPC], F32, tag="proj")
                nc.tensor.matmul(
                    wp[:], lhsT=warm[:, 0:128], rhs=warm[:],
                    start=True, stop=True,
                )

            qT = cpool.tile([128, RPC], BF16)
            kT = cpool.tile([128, RPC], BF16)

            def proj_qk(g):
                """q,k projection for graph g; evac q on vector, k on scalar.
                Returns (first_mm, last_mm) for PE-order shaping."""
                pq = pp.tile([128, RPC], F32, tag="proj")
                pk = pp.tile([128, RPC], F32, tag="proj")
                gs = slice(g * G, (g + 1) * G)
                first = last = None
                for s, p in ((0, pq), (1, pk)):
                    for ko in range(KO):
                        last = nc.tensor.matmul(
                            p[:, 0:G], lhsT=wsl(s, ko), rhs=xs[g][:, ko, :],
                            start=(ko == 0), stop=(ko == KO - 1),
                            skip_group_check=True,
                        )
                        first = first or last
                if with_bias:
                    nc.vector.tensor_scalar_add(qT[:, gs], pq[:, 0:G], bia[:, 0:1])
                    nc.scalar.activation(
                        kT[:, gs], pk[:, 0:G], ACT.Identity, bias=bia[:, 1:2]
                    )
                else:
                    nc.vector.tensor_copy(out=qT[:, gs], in_=pq[:, 0:G])
                    nc.scalar.activation(kT[:, gs], pk[:, 0:G], ACT.Identity)
                return first, last

            def proj_v(jt):
                """v projection for row-tile jt (128 rows)."""
                g = jt // 2
                lj = jt % 2
                pv = pvp.tile([128, DQ], F32, tag="vn")
                first = None
                for ko in range(KO):
                    mi = nc.tensor.matmul(
                        pv[:],
                        lhsT=xs[g][:, ko, lj * 128:(lj + 1) * 128],
                        rhs=wsl(2, ko),
                        start=(ko == 0), stop=(ko == KO - 1),
                    )
                    first = first or mi
                nc.vector.tensor_copy(out=vna[:, jt, 0:DQ], in_=pv[:])
                return first, mi

            eqs = [None, None]

            def scores_graph(g):
                """qk scores + bcm via identity-matmul; exp evacuated in
                column quarters so the PV matmuls can start early."""
                spg = ps.tile([128, 2 * G], F32, tag="s")  # 1 bank, both j-blocks
                first = None
                for jb in range(2):
                    t = 2 * g + jb
                    mi = nc.tensor.matmul(
                        spg[:, jb * G:(jb + 1) * G],
                        lhsT=kT[:, t * 128:(t + 1) * 128],
                        rhs=qT[:, g * G:(g + 1) * G],
                        start=(jb == 0), stop=False,
                        skip_group_check=True,
                    )
                    first = first or mi
                last = None
                for jb in range(2):
                    last = nc.tensor.matmul(
                        spg[:, jb * G:(jb + 1) * G],
                        lhsT=idn,
                        rhs=bcs[g][:, jb * G:(jb + 1) * G],
                        start=False, stop=(jb == 1),
                        skip_group_check=True,
                    )
                eq = epool.tile([128, 2 * G], BF16, tag="eq")
                # quarter order matches PV consumption: (rb0,jb0) (rb0,jb1)
                # (rb1,jb0) (rb1,jb1)
                for rb in range(2):
                    for jb in range(2):
                        qs = slice(jb * G + rb * 128, jb * G + rb * 128 + 128)
                        nc.scalar.activation(eq[:, qs], spg[:, qs], ACT.Exp)
                eqs[g] = eq
                return first, last

            out_sb = out_sb_t.ap()

            def pv_graph(g):
                """PV matmuls (+denominator column), one PSUM bank per
                row-tile so each half evacuates while the other accumulates;
                the store to HBM happens post-context, hidden under the
                NEFF's semaphore-clear postamble."""
                first = None
                for rb in range(2):
                    op = po.tile([128, VA], F32, tag="o")
                    for jb in range(2):
                        mi = nc.tensor.matmul(
                            op[:],
                            lhsT=eqs[g][:, jb * G + rb * 128: jb * G + rb * 128 + 128],
                            rhs=vna[:, 2 * g + jb, :],
                            start=(jb == 0), stop=(jb == 1),
                            skip_group_check=True,
                        )
                        first = first or mi
                    nc.vector.tensor_copy(
                        out=out_sb[:, 2 * g + rb, :], in_=op[:]
                    )
                return first, mi

            qk0 = proj_qk(0)
            qk1 = proj_qk(1)
            sc0 = scores_graph(0)
            v0 = proj_v(0)
            v1 = proj_v(1)
            pv0 = pv_graph(0)
            v2 = proj_v(2)
            v3 = proj_v(3)
            sc1 = scores_graph(1)
            pv1 = pv_graph(1)
            # Static PE order: qk g0, qk g1, scores g0, v g0..g1, pv g0,
            # v g1 tiles, scores g1, pv g1.  Keeps the scalar-engine evac of
            # k-g1 off the critical tail and the g0 chain ahead of g1's.
            order = [
                (sc0[0], qk1[1], "scores g0 after qk g1"),
                (v0[0], sc0[1], "v g0 after scores g0"),
                (v1[0], v0[1], "v jt1 after jt0"),
                (pv0[0], v1[1], "pv g0 after v g0"),
                (v2[0], pv0[1], "v jt2 after pv g0"),
                (v3[0], v2[1], "v jt3 after jt2"),
                (sc1[0], v3[1], "scores g1 after v g1"),
                (pv1[0], sc1[1], "pv g1 after scores g1"),
            ]
            for a, b, why in order:
                tile.add_dep_helper(a.ins, b.ins, sync=False, reason=why)
    # The tile-context exit barrier guarantees the out_sb evacs are complete;
    # the store's transfer + completion then overlap the fixed ~7us NEFF
    # semaphore-clear postamble instead of extending the critical path.
    # Walrus requires sync info on every dynamic DMA; nothing waits on it.
    out_sem = nc.alloc_semaphore("out_dma_sem")
    nc.scalar.dma_start(out_d, out_sb_t.ap()).then_inc(out_sem, 16)
    nc.compile()
    return nc


def get_nc(with_bias: bool) -> bass.Bass:
    key = f"nc{int(with_bias)}"
    if key not in _CACHE:
        _CACHE[key] = build_nc(with_bias)
    return _CACHE[key]


def make_in_maps(x, b, c, ptr, sparse_mask, Wq, bq, Wk, bk, Wv, bv, with_bias):
    """Host-side sharding: slice the block-diagonal, combine b+c with the mask
    sentinel, cast everything to bf16, transpose to partition-major layouts."""
    x = np.asarray(x, dtype=np.float32)
    b = np.asarray(b, dtype=np.float32)
    c = np.asarray(c, dtype=np.float32)
    ptr = np.asarray(ptr)
    mask = np.asarray(sparse_mask) != 0
    # fold 1/sqrt(dq) into Wq/bq so scores come out pre-scaled
    wq3 = (np.asarray(Wq).T * SCALE).astype(np.float32)
    wk3 = np.asarray(Wk).T.astype(np.float32)
    wv3 = np.asarray(Wv).T.astype(np.float32)  # each [DIN, DQ]

    assert np.array_equal(
        np.asarray(ptr).ravel(), np.arange(NG + 1) * G
    ), "kernel compiled for uniform 256-node graphs"

    def wshape(w3):  # [128, KO*DQ], partition-major over DIN
        return np.ascontiguousarray(
            w3.reshape(KO, 128, DQ).transpose(1, 0, 2)
        ).astype(BF).reshape(128, KO * DQ)

    wqkh = np.ascontiguousarray(
        np.concatenate([wshape(wq3), wshape(wk3)], axis=1)
    )  # [128, WQK]
    wvih = np.ascontiguousarray(
        np.concatenate([wshape(wv3), np.eye(128, dtype=BF)], axis=1)
    )  # [128, WVI]

    in_maps = []
    for i in range(NCORES):
        lo = i * RPC
        xT = x[lo:lo + RPC].T  # [DIN, RPC]
        xh = np.ascontiguousarray(
            xT.reshape(KO, 128, RPC).transpose(1, 0, 2)
        ).astype(BF)  # [128, KO, RPC]
        im = {"wqk": wqkh, "wvi": wvih}
        if with_bias:
            im["bias"] = np.ascontiguousarray(
                np.stack([np.asarray(bq) * SCALE, np.asarray(bk)], axis=1)
            ).astype(np.float32)
        for g in range(GPC):
            gs = slice(g * G, (g + 1) * G)
            im[f"x{g}"] = np.ascontiguousarray(xh[:, :, gs])
            blk = slice(lo + g * G, lo + (g + 1) * G)
            m = np.where(mask[blk, blk], b[blk, blk] + c[blk, blk], NEG).T
            # bc[p, jb*G + r] = m[jb*128+p, r]
            im[f"bc{g}"] = np.ascontiguousarray(
                m.reshape(2, 128, G).transpose(1, 0, 2).reshape(128, 2 * G)
            ).astype(BF)
        in_maps.append(im)
    return in_maps


def run(inputs: dict, trace: bool = False):
    """Run on all 8 cores; returns (full_output, BassKernelResults)."""
    bq = np.asarray(inputs["bq"], dtype=np.float32)
    bk = np.asarray(inputs["bk"], dtype=np.float32)
    with_bias = bool(np.any(bq) or np.any(bk))
    nc = get_nc(with_bias)
    in_maps = make_in_maps(**inputs, with_bias=with_bias)
    res = run_bass_kernel_spmd(
        nc, in_maps, core_ids=list(range(NCORES)), trace=trace
    )
    bv = np.asarray(inputs["bv"], dtype=np.float32)
    outs = []
    for r in res.results:
        o = np.asarray(r["out"]).astype(np.float32)  # [128, NT, VA]
        o = o[:, :, 0:DQ] / o[:, :, DQ:VA] + bv  # host-side norm + v bias
        outs.append(o.transpose(1, 0, 2).reshape(RPC, DQ))
    out = np.concatenate(outs, axis=0)
    return out, res


def kernel(**inputs) -> np.ndarray:
    out, _ = run(inputs, trace=False)
    return out


# revision 19
# speedup vs baseline: 1.2440x; 1.2440x over previous
"""Trainium2 Bass kernel for block-diagonal sparse attention (8 NeuronCores SPMD).

Problem: nn_AttentionHead (N=4096, DIM_IN=512, DQ=DK=128, 16 graphs of 256 nodes).
  q = x@Wq.T+bq; k = x@Wk.T+bk; v = x@Wv.T+bv
  a = where(block, qk/sqrt(dq), 0) + b + c; masked-softmax over block-diagonal
  out = (softmax(a)*keep) @ v

Key structural facts exploited:
  - Only the 16 diagonal 256x256 tiles of b/c/sparse_mask matter; the host
    slices them, combines bcm = b+c (masked entries -> -200 so exp gives 0),
    casts to bf16. HBM traffic is ~1.2MB/core instead of ~200MB.
  - Graphs are independent -> 2 graphs per core across 8 cores, zero cross-core
    communication (weights replicated).
  - The single per-core DMA engine drains the sync HW queue before the scalar
    HW queue, so inputs are laid out across the two queues in exact dependency
    order: [wqk | x-g0 | wv+I | x-g1] then [bc-g0 | bc-g1].  Compute streams
    behind the transfer instead of waiting for all input; each DMA trigger
    costs ~0.7us of engine time, so transfers are kept few and large.
  - bcm is added into the score PSUM by the PE itself via an identity-matmul
    accumulated onto the qk matmul, so the only post-processing is a single
    exp per graph straight out of the (single-bank) PSUM tile.
  - The denominator is obtained free by appending a ones-column to v in the PV
    matmul; the division happens on the HOST (outputs leave the chip
    unnormalized as [num | den] rows in bf16).
  - q/k/v biases never touch the chip when they are all zero (the actual
    inputs): out = num/den + bv is exact because sm @ (v0 + 1*bv^T) =
    sm@v0 + den*bv^T, and the bq/bk terms only shift softmax rows by
    constants.  A nonzero-bias graph variant is compiled only if needed.
  - 1/sqrt(dq) is folded into Wq host-side; everything is pre-cast to bf16.
  - The PE HAM clock-gate unthrottles 1.2->2.4GHz only after ~4us of gapless
    matmul activity, so narrow dummy warmup matmuls bridge the input-DMA
    phase at fine granularity; the real matmuls then run at full clock.
  - Per-graph pipelining: graph 0's projections/scores/exp/PV run while graph
    1's x/bc are still in flight.
"""

import math

import numpy as np
import ml_dtypes

import concourse.bass as bass
import concourse.mybir as mybir
import concourse.tile as tile
from concourse import bacc
from concourse.bass_utils import run_bass_kernel_spmd

# -------- problem constants (hardcoded per spec) --------
N = 4096
DIN = 512
DQ = 128           # == DK
NG = 16            # number of graphs
G = N // NG        # 256 nodes per graph
NCORES = 8
RPC = N // NCORES  # 512 rows per core
GPC = NG // NCORES  # 2 graphs per core
NT = RPC // 128    # 4 row-tiles of 128 per core
KO = DIN // 128    # 4 contraction tiles for the projections
VA = DQ + 1        # v augmented with a ones column (denominator trick)
SCALE = 1.0 / math.sqrt(DQ)
NEG = -200.0       # masked-entry sentinel; exp(-200 + |qk|max) == 0 in bf16
NWARM = 9          # wide PE HAM warmup matmuls (bridge to ~data arrival)

F32 = mybir.dt.float32
BF16 = mybir.dt.bfloat16

ACT = mybir.ActivationFunctionType
ALU = mybir.AluOpType

BF = ml_dtypes.bfloat16

WQK = 2 * KO * DQ        # wq | wk columns
WVI = KO * DQ + 128      # wv | identity columns

_CACHE: dict = {}


def build_nc(with_bias: bool) -> bass.Bass:
    """Build the per-core Bass graph (identical on all 8 cores)."""
    nc = bacc.Bacc(
        "TRN2",
        target_bir_lowering=False,
        debug=False,
        enable_asserts=False,
        num_devices=NCORES,
    )
    wqk_d = nc.dram_tensor("wqk", [128, WQK], BF16, kind="ExternalInput").ap()
    wvi_d = nc.dram_tensor("wvi", [128, WVI], BF16, kind="ExternalInput").ap()
    x_d = [
        nc.dram_tensor(f"x{g}", [128, KO, G], BF16, kind="ExternalInput").ap()
        for g in range(GPC)
    ]
    bc_d = [
        nc.dram_tensor(f"bc{g}", [128, 2 * G], BF16, kind="ExternalInput").ap()
        for g in range(GPC)
    ]
    if with_bias:
        bia_d = nc.dram_tensor("bias", [DQ, 2], F32, kind="ExternalInput").ap()
    out_d = nc.dram_tensor("out", [128, NT, VA], BF16, kind="ExternalOutput").ap()
    out_sb_t = nc.alloc_sbuf_tensor("out_sb", [128, NT, VA], BF16)

    with tile.TileContext(nc) as tc:
        with (
            tc.tile_pool(name="const", bufs=1) as cpool,
            tc.tile_pool(name="eq", bufs=2) as epool,
            tc.tile_pool(name="ps_proj", bufs=2, space="PSUM") as pp,
            tc.tile_pool(name="ps_v", bufs=2, space="PSUM") as pvp,
            tc.tile_pool(name="ps_s", bufs=2, space="PSUM") as ps,
            tc.tile_pool(name="ps_o", bufs=2, space="PSUM") as po,
        ):
            # warm tile on gpsimd (its preamble finishes first) so the PE
            # warmup starts as early as possible; only the lhsT columns need
            # defined data -- the rhs may read stale SBUF
            warm = cpool.tile([128, RPC], BF16)
            nc.gpsimd.memset(warm[:, 0:128], 1.0)

            # ---- input DMAs; the single DMA engine round-robins between the
            # two HW queues, so the effective arrival order is the zipper of
            # the two queue sequences: wqk||x0 first (enables the g0 q/k
            # projections), then wvi||x1, then bc0/bc1 for the score adds ----
            wqk = cpool.tile([128, WQK], BF16)
            nc.sync.dma_start(wqk[:], wqk_d)
            xs = [cpool.tile([128, KO, G], BF16, name=f"x{g}") for g in range(GPC)]
            nc.scalar.dma_start(xs[0][:], x_d[0])
            bcs = [
                cpool.tile([128, 2 * G], BF16, name=f"bc{g}") for g in range(GPC)
            ]
            nc.sync.dma_start(bcs[0][:], bc_d[0])
            wvi = cpool.tile([128, WVI], BF16)
            nc.sync.dma_start(wvi[:], wvi_d)
            nc.scalar.dma_start(xs[1][:], x_d[1])
            nc.scalar.dma_start(bcs[1][:], bc_d[1])
            if with_bias:
                bia = cpool.tile([128, 2], F32)
                nc.scalar.dma_start(bia[:], bia_d)

            def wsl(s, ko):  # weight slice for projection s, contraction ko
                if s < 2:
                    return wqk[:, (s * KO + ko) * DQ:(s * KO + ko + 1) * DQ]
                return wvi[:, ko * DQ:(ko + 1) * DQ]

            idn = wvi[:, KO * DQ:KO * DQ + 128]

            vna = cpool.tile([128, NT, VA], BF16)  # [j%128, j//128, d | 1]
            nc.vector.memset(vna[:, :, DQ:VA], 1.0)

            # ---- PE HAM warmup: narrow matmuls so real work preempts the
            # bridge at fine granularity once its data lands ----
            for _ in range(NWARM):
                wp = pp.tile([128, RPC], F32, tag="proj")
                nc.tensor.matmul(
                    wp[:], lhsT=warm[:, 0:128], rhs=warm[:],
                    start=True, stop=True,
                )

            qT = cpool.tile([128, RPC], BF16)
            kT = cpool.tile([128, RPC], BF16)

            def proj_qk(g):
                """q,k projection for graph g into one shared PSUM bank
                (region-granular accumulation groups); evac q on vector, k on
                scalar.  Returns (first_mm, last_mm) for PE-order shaping."""
                pqk = pp.tile([128, 2, G], F32, tag="proj")
                gs = slice(g * G, (g + 1) * G)
                first = last = None
                for s in (0, 1):
                    for ko in range(KO):
                        last = nc.tensor.matmul(
                            pqk[:, s, :], lhsT=wsl(s, ko), rhs=xs[g][:, ko, :],
                            start=(ko == 0), stop=(ko == KO - 1),
                            skip_group_check=True,
                        )
                        first = first or last
                if with_bias:
                    nc.vector.tensor_scalar_add(
                        qT[:, gs], pqk[:, 0, :], bia[:, 0:1]
                    )
                    nc.scalar.activation(
                        kT[:, gs], pqk[:, 1, :], ACT.Identity, bias=bia[:, 1:2]
                    )
                else:
                    nc.vector.tensor_copy(out=qT[:, gs], in_=pqk[:, 0, :])
                    nc.scalar.activation(kT[:, gs], pqk[:, 1, :], ACT.Identity)
                return first, last

            def proj_v(jt):
                """v projection for row-tile jt (128 rows)."""
                g = jt // 2
                lj = jt % 2
                pv = pvp.tile([128, DQ], F32, tag="vn")
                first = None
                for ko in range(KO):
                    mi = nc.tensor.matmul(
                        pv[:],
                        lhsT=xs[g][:, ko, lj * 128:(lj + 1) * 128],
                        rhs=wsl(2, ko),
                        start=(ko == 0), stop=(ko == KO - 1),
                    )
                    first = first or mi
                nc.vector.tensor_copy(out=vna[:, jt, 0:DQ], in_=pv[:])
                return first, mi

            eqs = [None, None]

            def scores_graph(g):
                """qk scores + bcm via identity-matmul; exp evacuated in
                column quarters so the PV matmuls can start early."""
                spg = ps.tile([128, 2 * G], F32, tag="s")  # 1 bank, both j-blocks
                first = None
                for jb in range(2):
                    t = 2 * g + jb
                    mi = nc.tensor.matmul(
                        spg[:, jb * G:(jb + 1) * G],
                        lhsT=kT[:, t * 128:(t + 1) * 128],
                        rhs=qT[:, g * G:(g + 1) * G],
                        start=(jb == 0), stop=False,
                        skip_group_check=True,
                    )
                    first = first or mi
                last = None
                for jb in range(2):
                    last = nc.tensor.matmul(
                        spg[:, jb * G:(jb + 1) * G],
                        lhsT=idn,
                        rhs=bcs[g][:, jb * G:(jb + 1) * G],
                        start=False, stop=(jb == 1),
                        skip_group_check=True,
                    )
                eq = epool.tile([128, 2 * G], BF16, tag="eq")
                nc.scalar.activation(eq[:], spg[:], ACT.Exp)
                eqs[g] = eq
                return first, last

            out_sb = out_sb_t.ap()

            def pv_graph(g):
                """PV matmuls (+denominator column), one PSUM bank per
                row-tile so each half evacuates while the other accumulates;
                the store to HBM happens post-context, hidden under the
                NEFF's semaphore-clear postamble."""
                first = None
                for rb in range(2):
                    op = po.tile([128, VA], F32, tag="o")
                    for jb in range(2):
                        mi = nc.tensor.matmul(
                            op[:],
                            lhsT=eqs[g][:, jb * G + rb * 128: jb * G + rb * 128 + 128],
                            rhs=vna[:, 2 * g + jb, :],
                            start=(jb == 0), stop=(jb == 1),
                            skip_group_check=True,
                        )
                        first = first or mi
                    nc.vector.tensor_copy(
                        out=out_sb[:, 2 * g + rb, :], in_=op[:]
                    )
                return first, mi

            qk0 = proj_qk(0)
            qk1 = proj_qk(1)
            sc0 = scores_graph(0)
            v0 = proj_v(0)
            v1 = proj_v(1)
            pv0 = pv_graph(0)
            v2 = proj_v(2)
            v3 = proj_v(3)
            sc1 = scores_graph(1)
            pv1 = pv_graph(1)
            # Light PE-order shaping: run qk g1 right after qk g0 (so its
            # k-evac lands early), and keep the v projections from jumping
            # ahead of graph 0's score chain.
            order = [
                (sc0[0], qk1[1], "scores g0 after qk g1"),
                (v0[0], sc0[1], "v g0 after scores g0"),
            ]
            for a, b, why in order:
                tile.add_dep_helper(a.ins, b.ins, sync=False, reason=why)
    # The tile-context exit barrier guarantees the out_sb evacs are complete;
    # the store's transfer + completion then overlap the fixed ~7us NEFF
    # semaphore-clear postamble instead of extending the critical path.
    # Walrus requires sync info on every dynamic DMA; nothing waits on it.
    out_sem = nc.alloc_semaphore("out_dma_sem")
    nc.scalar.dma_start(out_d, out_sb_t.ap()).then_inc(out_sem, 16)
    nc.compile()
    return nc


def get_nc(with_bias: bool) -> bass.Bass:
    key = f"nc{int(with_bias)}"
    if key not in _CACHE:
        _CACHE[key] = build_nc(with_bias)
    return _CACHE[key]


def make_in_maps(x, b, c, ptr, sparse_mask, Wq, bq, Wk, bk, Wv, bv, with_bias):
    """Host-side sharding: slice the block-diagonal, combine b+c with the mask
    sentinel, cast everything to bf16, transpose to partition-major layouts."""
    x = np.asarray(x, dtype=np.float32)
    b = np.asarray(b, dtype=np.float32)
    c = np.asarray(c, dtype=np.float32)
    ptr = np.asarray(ptr)
    mask = np.asarray(sparse_mask) != 0
    # fold 1/sqrt(dq) into Wq/bq so scores come out pre-scaled
    wq3 = (np.asarray(Wq).T * SCALE).astype(np.float32)
    wk3 = np.asarray(Wk).T.astype(np.float32)
    wv3 = np.asarray(Wv).T.astype(np.float32)  # each [DIN, DQ]

    assert np.array_equal(
        np.asarray(ptr).ravel(), np.arange(NG + 1) * G
    ), "kernel compiled for uniform 256-node graphs"

    def wshape(w3):  # [128, KO*DQ], partition-major over DIN
        return np.ascontiguousarray(
            w3.reshape(KO, 128, DQ).transpose(1, 0, 2)
        ).astype(BF).reshape(128, KO * DQ)

    wqkh = np.ascontiguousarray(
        np.concatenate([wshape(wq3), wshape(wk3)], axis=1)
    )  # [128, WQK]
    wvih = np.ascontiguousarray(
        np.concatenate([wshape(wv3), np.eye(128, dtype=BF)], axis=1)
    )  # [128, WVI]

    in_maps = []
    for i in range(NCORES):
        lo = i * RPC
        xT = x[lo:lo + RPC].T  # [DIN, RPC]
        xh = np.ascontiguousarray(
            xT.reshape(KO, 128, RPC).transpose(1, 0, 2)
        ).astype(BF)  # [128, KO, RPC]
        im = {"wqk": wqkh, "wvi": wvih}
        if with_bias:
            im["bias"] = np.ascontiguousarray(
                np.stack([np.asarray(bq) * SCALE, np.asarray(bk)], axis=1)
            ).astype(np.float32)
        for g in range(GPC):
            gs = slice(g * G, (g + 1) * G)
            im[f"x{g}"] = np.ascontiguousarray(xh[:, :, gs])
            blk = slice(lo + g * G, lo + (g + 1) * G)
            m = np.where(mask[blk, blk], b[blk, blk] + c[blk, blk], NEG).T
            # bc[p, jb*G + r] = m[jb*128+p, r]
            im[f"bc{g}"] = np.ascontiguousarray(
                m.reshape(2, 128, G).transpose(1, 0, 2).reshape(128, 2 * G)
            ).astype(BF)
        in_maps.append(im)
    return in_maps


def run(inputs: dict, trace: bool = False):
    """Run on all 8 cores; returns (full_output, BassKernelResults)."""
    bq = np.asarray(inputs["bq"], dtype=np.float32)
    bk = np.asarray(inputs["bk"], dtype=np.float32)
    with_bias = bool(np.any(bq) or np.any(bk))
    nc = get_nc(with_bias)
    in_maps = make_in_maps(**inputs, with_bias=with_bias)
    res = run_bass_kernel_spmd(
        nc, in_maps, core_ids=list(range(NCORES)), trace=trace
    )
    bv = np.asarray(inputs["bv"], dtype=np.float32)
    outs = []
    for r in res.results:
        o = np.asarray(r["out"]).astype(np.float32)  # [128, NT, VA]
        o = o[:, :, 0:DQ] / o[:, :, DQ:VA] + bv  # host-side norm + v bias
        outs.append(o.transpose(1, 0, 2).reshape(RPC, DQ))
    out = np.concatenate(outs, axis=0)
    return out, res


def kernel(**inputs) -> np.ndarray:
    out, _ = run(inputs, trace=False)
    return out
